# revision 6
# baseline (speedup 1.0000x reference)
"""Trainium2 Bass kernel for nn_CLIP topk_masking.

Computes, for full inputs (self-contained; shapes hardcoded):
    probability = image_features @ ima_proto.T          # [B, NP]
    thr_r       = k-th largest of probability row r
    sel[r, j]   = probability[r, j] >= thr_r            # top-k prototype mask
    text_n      = exp(logit_scale) * text_raw / ||text_raw||_row
    logits[r,c] = (image_features @ text_n.T)[r,c] * sel[r, c // 10]

Sharding: data-parallel over the batch axis across 8 NeuronCores
(rows 512/core); prototypes and text features replicated.

Design notes (v2 rewrite):
  - All matmul operands are staged HOST-SIDE in transposed, DMA-optimal
    tiled layouts (imgT / imgT_h / protoT / textT).  This removes every
    PE transpose and every PSUM->SBUF transpose copy of the previous
    version; the PE runs only the two real matmuls plus one tiny
    norm-reduction matmul per text chunk.
  - Text norms in the transposed layout: scalar engine squares each
    chunk (fp16), DVE combines the 4 contraction tiles, and a single
    ones[128,128] matmul both reduces over the last 128 partitions AND
    replicates norm^2 across partitions (PSUM).  Scalar does
    sqrt(norm2 * exp(-2s)); DVE reciprocal yields u_rep[128, 500] =
    exp(s)/||text_c|| replicated across partitions.
  - The mask is built on GPSIMD with one fused scalar_tensor_tensor per
    (row-tile, chunk): selxu = (prob >= thr) * u_rep.  This fuses the
    top-k compare, the block->class broadcast, and the norm scale.
  - DVE applies logits = pm * selxu straight out of PSUM into the fp16
    staging tile; SWDGE stores per 2000-column group (final group per
    chunk so the tail drains early).
  - Output is stored fp16 and upcast on the host.
"""

import os
from contextlib import ExitStack

import numpy as np

import concourse.bass as bass
import concourse.tile as tile
from concourse import bacc, mybir
from concourse.bass_utils import run_bass_kernel_spmd

# Problem shapes (hardcoded per contract).
B, D, NP, NC, CPT = 4096, 512, 1000, 10000, 10
NCORES = 8
RLOC = B // NCORES          # 512 rows per core
RT = RLOC // 128            # 4 row tiles per core
KD = D // 128               # 4 contraction chunks
CHW = 500                   # classes per chunk (matmul N; = 50 proto blocks)
NCH = NC // CHW             # 20 chunks
GRP = 4                     # chunks per group (per text-load / out-store DMA)
NG = NCH // GRP             # 5 groups
BPC = CHW // CPT            # 50 proto blocks per chunk
NEG = -1.0e30

F32 = mybir.dt.float32
F16 = mybir.dt.float16

LAST_RESULTS = None


def _emit(ctx: ExitStack, tc, imgT_d, imgTh_d, protoT_d, textT_d, out_d,
          k: int, inv_s2: float):
    nc = tc.nc
    AF = mybir.ActivationFunctionType
    OP = mybir.AluOpType

    const = ctx.enter_context(tc.tile_pool(name="const", bufs=1))
    persist = ctx.enter_context(tc.tile_pool(name="persist", bufs=1))

    ones_h = const.tile([128, 128], F16)
    nc.vector.memset(ones_h[:], 1.0)

    imgT = persist.tile([128, KD, RLOC], F32)
    imgTh = persist.tile([128, KD, RLOC], F16)
    protoT = persist.tile([128, KD, NP], F32)
    probs = [persist.tile([128, NP], F32, tag=f"prob{rt}", name=f"prob{rt}")
             for rt in range(RT)]

    # SWDGE load order: protoT + imgT first (phase-A critical path), then
    # text groups.  imgT_h rides the scalar HWDGE ring (disjoint engine
    # pressure).  protoT split in kc-halves so prob matmuls could start
    # on the first half while the second lands.
    nc.gpsimd.dma_start(protoT[:, :2], protoT_d[:, :2])
    nc.gpsimd.dma_start(protoT[:, 2:], protoT_d[:, 2:])
    nc.gpsimd.dma_start(imgT[:], imgT_d)
    nc.scalar.dma_start(imgTh[:], imgTh_d)

    pb_text = ctx.enter_context(tc.tile_pool(name="pb_text", bufs=3))
    pb_sq = ctx.enter_context(tc.tile_pool(name="pb_sq", bufs=2))
    pb_s = ctx.enter_context(tc.tile_pool(name="pb_s", bufs=6))
    pb_u = ctx.enter_context(tc.tile_pool(name="pb_u", bufs=4))
    pb_sx = ctx.enter_context(tc.tile_pool(name="pb_sx", bufs=10))
    pb_stage = ctx.enter_context(tc.tile_pool(name="pb_stage", bufs=2))
    pa_work = ctx.enter_context(tc.tile_pool(name="pa_work", bufs=2))
    pb_pace = ctx.enter_context(tc.tile_pool(name="pb_pace", bufs=2))

    tts = {}

    def pace_on(src_ap):
        # 1-element gpsimd read: delays subsequent SWDGE descriptor
        # generation until `src_ap`'s DMA completes, so earlier loads
        # keep all 16 DMA engines to themselves.
        pace = pb_pace.tile([1, 2], F32, tag="pace")
        nc.gpsimd.tensor_copy(pace[:].rearrange("a (b c) -> a b c", b=1), src_ap)

    def load_group(g: int):
        t_ = pb_text.tile([128, KD, GRP * CHW], F16, name=f"tt{g}", tag="tt")
        nc.gpsimd.dma_start(t_[:], textT_d[:, g])
        tts[g] = t_

    pace_on(protoT[0:1, 3:4, 0:2])
    pace_on(imgT[0:1, 0:1, 0:2])
    load_group(0)
    load_group(1)

    # ---------- Phase A: probability matmuls (f32) + per-row top-k thr ----
    thrs = []
    with tc.tile_pool(name="pa_ps", bufs=2, space="PSUM") as pa_ps:
        for rt in range(RT):
            for h in range(2):
                ppr = pa_ps.tile([128, NP // 2], F32, tag="ppr")
                for kc in range(KD):
                    # fp32 (not fp16): ranking precision decides the mask.
                    nc.tensor.matmul(
                        ppr[:],
                        imgT[:, kc, rt * 128:(rt + 1) * 128],
                        protoT[:, kc, h * (NP // 2):(h + 1) * (NP // 2)],
                        start=(kc == 0), stop=(kc == KD - 1),
                    )
                nc.scalar.copy(
                    probs[rt][:, h * (NP // 2):(h + 1) * (NP // 2)], ppr[:])
            # top-k threshold right behind each row-tile's prob so the
            # first mask ops unblock as early as possible.
            m8a = persist.tile([128, 8], F32, tag=f"m8a{rt}")
            nc.vector.max(m8a[:], probs[rt][:])
            if k <= 8:
                thrs.append(m8a[:, k - 1:k])
            else:
                repl = pa_work.tile([128, NP], F32, tag="repl")
                nc.vector.match_replace(repl[:], m8a[:], probs[rt][:], NEG)
                m8b = persist.tile([128, 8], F32, tag=f"m8b{rt}")
                nc.vector.max(m8b[:], repl[:])
                thrs.append(m8b[:, k - 9:k - 8])

    # ---------- Phase B: norms, logit matmuls, fused mask, store ----------
    with (
        tc.tile_pool(name="pb_psM", bufs=5, space="PSUM") as pb_psM,
        tc.tile_pool(name="pb_psN", bufs=2, space="PSUM") as pb_psN,
    ):
        ttns = {}

        def norm_scale_chunk(c: int):
            # tt_n = tt * exp(s)/||text_col||  (u replicated over partitions)
            g, pos = divmod(c, GRP)
            tt = tts[g]
            sq = pb_sq.tile([128, KD, CHW], F16, tag="sq")
            nc.scalar.activation(
                sq[:], tt[:, :, pos * CHW:(pos + 1) * CHW], AF.Square)
            s01 = pb_s.tile([128, CHW], F16, tag="s01")
            s23 = pb_s.tile([128, CHW], F16, tag="s23")
            with nc.allow_low_precision(
                    reason="fp16 partial sums of squares: rel err ~5e-4, "
                    "well inside the 2e-2 gate"):
                nc.vector.tensor_tensor(s01[:], sq[:, 0], sq[:, 1], op=OP.add)
                nc.vector.tensor_tensor(s23[:], sq[:, 2], sq[:, 3], op=OP.add)
            # ones.T @ s01 + ones.T @ s23: reduces the last 128 partitions
            # AND replicates norm^2 across all 128 output partitions.
            nr = pb_psN.tile([128, CHW], F32, tag="nr")
            nc.tensor.matmul(nr[:], ones_h[:], s01[:], start=True, stop=False)
            nc.tensor.matmul(nr[:], ones_h[:], s23[:], start=False, stop=True)
            # sqrt(norm2 * exp(-2s)) = ||t|| / s ; then reciprocal.
            nrs = pb_u.tile([128, CHW], F16, tag="nrs")
            nc.scalar.activation(nrs[:], nr[:], AF.Sqrt, scale=inv_s2)
            ur = pb_u.tile([128, CHW], F16, tag="ur")
            with nc.allow_low_precision(
                    reason="fp16 u = exp(s)/||t||: rel err ~5e-4, well "
                    "inside the 2e-2 gate"):
                nc.vector.reciprocal(ur[:], nrs[:])
            ttn = pb_sx.tile([128, KD, CHW], F16, tag="ttn")
            for kc in range(KD):
                eng = nc.vector if kc == 0 else nc.gpsimd
                eng.tensor_tensor(
                    ttn[:, kc], tt[:, kc, pos * CHW:(pos + 1) * CHW], ur[:],
                    op=OP.mult)
            ttns[c] = ttn

        stages = {}
        norm_scale_chunk(0)
        norm_scale_chunk(1)
        for c in range(NCH):
            g, pos = divmod(c, GRP)
            if pos == 0 and g + 2 < NG:
                pace_on(tts[g][0:1, 0:1, 0:2])
                load_group(g + 2)
            # Norm/scale pipeline two chunks ahead so tt_n never gates PE.
            if c + 2 < NCH:
                norm_scale_chunk(c + 2)
            ttn = ttns.pop(c)
            if pos == 0:
                stages[g] = pb_stage.tile(
                    [128, RT, GRP * CHW], F16, name=f"stg{g}", tag="stg")
            for rt in range(RT):
                pm = pb_psM.tile([128, CHW], F32, tag="pm")
                for kc in range(KD):
                    nc.tensor.matmul(
                        pm[:],
                        imgTh[:, kc, rt * 128:(rt + 1) * 128],
                        ttn[:, kc],
                        start=(kc == 0), stop=(kc == KD - 1),
                    )
                # Fused top-k mask + PSUM->SBUF move on DVE:
                #   stage = (prob >= thr) * pm    [block -> class bcast]
                nc.vector.scalar_tensor_tensor(
                    stages[g][:, rt, pos * CHW:(pos + 1) * CHW].rearrange(
                        "p (b o) -> p b o", o=CPT),
                    probs[rt][:, c * BPC:(c + 1) * BPC].broadcast_to(
                        [128, BPC, CPT]),
                    thrs[rt],
                    pm[:].rearrange("p (b o) -> p b o", o=CPT),
                    op0=OP.is_ge, op1=OP.mult)
            outv = out_d.rearrange("(t p) c -> p t c", p=128)
            if g == NG - 1:
                # Final group: store per-chunk so the tail drains early.
                nc.gpsimd.dma_start(
                    outv[:, :, g * GRP * CHW + pos * CHW:
                         g * GRP * CHW + (pos + 1) * CHW],
                    stages[g][:, :, pos * CHW:(pos + 1) * CHW])
            elif pos == GRP - 1:
                nc.gpsimd.dma_start(
                    outv[:, :, g * GRP * CHW:(g + 1) * GRP * CHW],
                    stages[g][:])


def _build(k: int, inv_s2: float):
    nc = bacc.Bacc("TRN2", target_bir_lowering=False, debug=False)
    imgT_d = nc.dram_tensor(
        "imgT", [128, KD, RLOC], F32, kind="ExternalInput").ap()
    imgTh_d = nc.dram_tensor(
        "imgTh", [128, KD, RLOC], F16, kind="ExternalInput").ap()
    protoT_d = nc.dram_tensor(
        "protoT", [128, KD, NP], F32, kind="ExternalInput").ap()
    textT_d = nc.dram_tensor(
        "textT", [128, NG, KD, GRP * CHW], F16, kind="ExternalInput").ap()
    out_d = nc.dram_tensor(
        "out", [RLOC, NC], F16, kind="ExternalOutput").ap()
    with tile.TileContext(nc) as tc:
        with ExitStack() as ctx:
            _emit(ctx, tc, imgT_d, imgTh_d, protoT_d, textT_d, out_d,
                  k, inv_s2)
    nc.compile()
    return nc


def _tileT(a: np.ndarray) -> np.ndarray:
    """[N, D] -> [128, KD, N] with [p, kc, n] = a[n, kc*128 + p]."""
    n = a.shape[0]
    return np.ascontiguousarray(
        a.T.reshape(KD, 128, n).transpose(1, 0, 2))


def kernel(image_features, ima_proto, text_features_raw, logit_scale, num_test):
    global LAST_RESULTS
    img = np.ascontiguousarray(np.asarray(image_features, dtype=np.float32))
    proto = np.ascontiguousarray(np.asarray(ima_proto, dtype=np.float32))
    text = np.ascontiguousarray(np.asarray(text_features_raw, dtype=np.float32))
    assert img.shape == (B, D) and proto.shape == (NP, D) and text.shape == (NC, D)
    s = float(np.asarray(logit_scale))
    k = min(int(np.asarray(num_test)), NP)
    assert 1 <= k <= 16, f"kernel supports k in [1, 16], got {k}"
    inv_s2 = float(np.exp(-2.0 * s))

    nc = _build(k, inv_s2)

    # Host-side layout staging (transposes + dtype only; all math on device).
    protoT = _tileT(proto)                                   # [128, KD, 1000]
    textT4 = _tileT(text.astype(np.float16))                 # [128, KD, 10000]
    textT = np.ascontiguousarray(
        textT4.reshape(128, KD, NG, GRP * CHW).transpose(0, 2, 1, 3))
    in_maps = []
    for i in range(NCORES):
        imgT = _tileT(img[i * RLOC:(i + 1) * RLOC])          # [128, KD, 512]
        in_maps.append({
            "imgT": imgT,
            "imgTh": imgT.astype(np.float16),
            "protoT": protoT,
            "textT": textT,
        })
    trace = bool(int(os.environ.get("BASS_KERNEL_TRACE", "0")))
    res = run_bass_kernel_spmd(nc, in_maps, list(range(NCORES)), trace=trace)
    LAST_RESULTS = res
    return np.concatenate(
        [r["out"].astype(np.float32) for r in res.results], axis=0)


# revision 9
# speedup vs baseline: 1.3060x; 1.3060x over previous
"""Trainium2 Bass kernel for nn_CLIP topk_masking.

Computes, for full inputs (self-contained; shapes hardcoded):
    probability = image_features @ ima_proto.T          # [B, NP]
    thr_r       = k-th largest of probability row r
    sel[r, j]   = probability[r, j] >= thr_r            # top-k prototype mask
    text_n      = exp(logit_scale) * text_raw / ||text_raw||_row
    logits[r,c] = (image_features @ text_n.T)[r,c] * sel[r, c // 10]

Sharding: data-parallel over the batch axis across 8 NeuronCores
(rows 512/core); prototypes and text features replicated.

Design notes (v2 rewrite):
  - All matmul operands are staged HOST-SIDE in transposed, DMA-optimal
    tiled layouts (imgT / imgT_h / protoT / textT).  This removes every
    PE transpose and every PSUM->SBUF transpose copy of the previous
    version; the PE runs only the two real matmuls plus one tiny
    norm-reduction matmul per text chunk.
  - Text norms in the transposed layout: scalar engine squares each
    chunk (fp16), DVE combines the 4 contraction tiles, and a single
    ones[128,128] matmul both reduces over the last 128 partitions AND
    replicates norm^2 across partitions (PSUM).  Scalar does
    sqrt(norm2 * exp(-2s)); DVE reciprocal yields u_rep[128, 500] =
    exp(s)/||text_c|| replicated across partitions.
  - The mask is built on GPSIMD with one fused scalar_tensor_tensor per
    (row-tile, chunk): selxu = (prob >= thr) * u_rep.  This fuses the
    top-k compare, the block->class broadcast, and the norm scale.
  - DVE applies logits = pm * selxu straight out of PSUM into the fp16
    staging tile; SWDGE stores per 2000-column group (final group per
    chunk so the tail drains early).
  - Output is stored fp16 and upcast on the host.
"""

import os
from contextlib import ExitStack

import numpy as np

import concourse.bass as bass
import concourse.tile as tile
from concourse import bacc, mybir
from concourse.bass_utils import run_bass_kernel_spmd

# Problem shapes (hardcoded per contract).
B, D, NP, NC, CPT = 4096, 512, 1000, 10000, 10
NCORES = 8
RLOC = B // NCORES          # 512 rows per core
RT = RLOC // 128            # 4 row tiles per core
KD = D // 128               # 4 contraction chunks
CHW = 500                   # classes per chunk (matmul N; = 50 proto blocks)
NCH = NC // CHW             # 20 chunks
GRP = 4                     # chunks per group (per text-load / out-store DMA)
NG = NCH // GRP             # 5 groups
BPC = CHW // CPT            # 50 proto blocks per chunk
NEG = -1.0e30

F32 = mybir.dt.float32
F16 = mybir.dt.float16

LAST_RESULTS = None


def _emit(ctx: ExitStack, tc, imgT_d, imgTh_d, protoT_d, textT_d, out_d,
          k: int, inv_s2: float):
    nc = tc.nc
    AF = mybir.ActivationFunctionType
    OP = mybir.AluOpType

    const = ctx.enter_context(tc.tile_pool(name="const", bufs=1))
    persist = ctx.enter_context(tc.tile_pool(name="persist", bufs=1))

    ones_h = const.tile([128, 128], F16)
    nc.vector.memset(ones_h[:], 1.0)

    imgT = persist.tile([128, KD, RLOC], F32)
    imgTh = persist.tile([128, KD, RLOC], F16)
    protoT = persist.tile([128, KD, NP], F32)
    probs = [persist.tile([128, NP], F32, tag=f"prob{rt}", name=f"prob{rt}")
             for rt in range(RT)]

    # SWDGE load order: protoT + imgT first (phase-A critical path), then
    # text groups.  imgT_h rides the scalar HWDGE ring (disjoint engine
    # pressure).  protoT split in kc-halves so prob matmuls could start
    # on the first half while the second lands.
    nc.gpsimd.dma_start(protoT[:, :2], protoT_d[:, :2])
    nc.gpsimd.dma_start(protoT[:, 2:], protoT_d[:, 2:])
    nc.gpsimd.dma_start(imgT[:], imgT_d)
    nc.scalar.dma_start(imgTh[:], imgTh_d)

    pb_text = ctx.enter_context(tc.tile_pool(name="pb_text", bufs=3))
    pb_sq = ctx.enter_context(tc.tile_pool(name="pb_sq", bufs=3))
    pb_u = ctx.enter_context(tc.tile_pool(name="pb_u", bufs=4))
    pb_sx = ctx.enter_context(tc.tile_pool(name="pb_sx", bufs=4))
    pb_stage = ctx.enter_context(tc.tile_pool(name="pb_stage", bufs=2))
    pa_work = ctx.enter_context(tc.tile_pool(name="pa_work", bufs=2))
    pb_pace = ctx.enter_context(tc.tile_pool(name="pb_pace", bufs=2))

    tts = {}

    def pace_on(src_ap):
        # 1-element gpsimd read: delays subsequent SWDGE descriptor
        # generation until `src_ap`'s DMA completes, so earlier loads
        # keep all 16 DMA engines to themselves.
        pace = pb_pace.tile([1, 2], F32, tag="pace")
        nc.gpsimd.tensor_copy(pace[:].rearrange("a (b c) -> a b c", b=1), src_ap)

    def load_group(g: int):
        t_ = pb_text.tile([128, KD, GRP * CHW], F16, name=f"tt{g}", tag="tt")
        nc.gpsimd.dma_start(t_[:], textT_d[:, g])
        tts[g] = t_

    pace_on(protoT[0:1, 3:4, 0:2])
    pace_on(imgT[0:1, 0:1, 0:2])
    load_group(0)
    load_group(1)

    # ---------- Phase A: probability matmuls (f32) + per-row top-k thr ----
    thrs = []
    with tc.tile_pool(name="pa_ps", bufs=2, space="PSUM") as pa_ps:
        for rt in range(RT):
            for h in range(2):
                ppr = pa_ps.tile([128, NP // 2], F32, tag="ppr")
                for kc in range(KD):
                    # fp32 (not fp16): ranking precision decides the mask.
                    nc.tensor.matmul(
                        ppr[:],
                        imgT[:, kc, rt * 128:(rt + 1) * 128],
                        protoT[:, kc, h * (NP // 2):(h + 1) * (NP // 2)],
                        start=(kc == 0), stop=(kc == KD - 1),
                    )
                nc.scalar.copy(
                    probs[rt][:, h * (NP // 2):(h + 1) * (NP // 2)], ppr[:])
            # top-k threshold right behind each row-tile's prob so the
            # first mask ops unblock as early as possible.
            m8a = persist.tile([128, 8], F32, tag=f"m8a{rt}")
            nc.vector.max(m8a[:], probs[rt][:])
            if k <= 8:
                thrs.append(m8a[:, k - 1:k])
            else:
                repl = pa_work.tile([128, NP], F32, tag="repl")
                nc.vector.match_replace(repl[:], m8a[:], probs[rt][:], NEG)
                m8b = persist.tile([128, 8], F32, tag=f"m8b{rt}")
                nc.vector.max(m8b[:], repl[:])
                thrs.append(m8b[:, k - 9:k - 8])

    # ---------- Phase B: norms, logit matmuls, fused mask, store ----------
    # (sq/ttn inner dim padded to 512 so every fp16 kc slice is 4-byte
    # aligned: misaligned rhs drops the PE to single-pump rate.)
    with (
        tc.tile_pool(name="pb_psM", bufs=3, space="PSUM") as pb_psM,
        tc.tile_pool(name="pb_psN", bufs=2, space="PSUM") as pb_psN,
    ):
        sqs, nrps, ttns = {}, {}, {}

        def norm_front(c: int):
            # squares (scalar) + partition-reduce-and-replicate (PE):
            # nr[p, j] = norm^2 of text column j, replicated over p.
            g, pos = divmod(c, GRP)
            sq = pb_sq.tile([128, KD, 512], F16, tag="sq")
            nc.scalar.activation(
                sq[:, :, :CHW], tts[g][:, :, pos * CHW:(pos + 1) * CHW],
                AF.Square)
            nr = pb_psN.tile([128, CHW], F32, tag="nr")
            for kc in range(KD):
                nc.tensor.matmul(
                    nr[:], ones_h[:], sq[:, kc, :CHW],
                    start=(kc == 0), stop=(kc == KD - 1))
            sqs[c] = sq
            nrps[c] = nr

        def norm_back(c: int):
            # sqrt(norm2 * exp(-2s)) = ||t||/s; approx-reciprocal; scale tt.
            g, pos = divmod(c, GRP)
            del sqs[c]
            nr = nrps.pop(c)
            nrs = pb_u.tile([128, CHW], F32, tag="nrs")
            nc.scalar.activation(nrs[:], nr[:], AF.Sqrt, scale=inv_s2)
            ur = pb_u.tile([128, CHW], F32, tag="ur")
            nc.vector.reciprocal_approx_fast(ur[:], nrs[:])
            ttn = pb_sx.tile([128, KD, 512], F16, tag="ttn")
            for kc in range(KD):
                eng = nc.vector if kc == 0 else nc.gpsimd
                eng.tensor_tensor(
                    ttn[:, kc, :CHW],
                    tts[g][:, kc, pos * CHW:(pos + 1) * CHW], ur[:],
                    op=OP.mult)
            ttns[c] = ttn

        stages = {}
        outv = out_d.rearrange("(t p) c -> p t c", p=128)
        norm_front(0)
        norm_front(1)
        norm_back(0)
        norm_back(1)
        for pr in range(NCH // 2):
            c0 = 2 * pr
            g, pos = divmod(c0, GRP)
            if pos == 0 and g + 2 < NG:
                pace_on(tts[g][0:1, 0:1, 0:2])
                load_group(g + 2)
            # Norm squares/reduce two pairs ahead of the PE.
            for cf in (c0 + 2, c0 + 3):
                if cf < NCH:
                    norm_front(cf)
            ttn0 = ttns.pop(c0)
            ttn1 = ttns.pop(c0 + 1)
            if pos == 0:
                stages[g] = pb_stage.tile(
                    [128, RT, GRP * CHW], F16, name=f"stg{g}", tag="stg")
            for rt in range(RT):
                pm = pb_psM.tile([128, 2, 512], F32, tag="pm")
                for side, ttn in ((0, ttn0), (1, ttn1)):
                    for kc in range(KD):
                        nc.tensor.matmul(
                            pm[:, side, :CHW],
                            imgTh[:, kc, rt * 128:(rt + 1) * 128],
                            ttn[:, kc, :CHW],
                            start=(kc == 0), stop=(kc == KD - 1),
                        )
                # Fused top-k mask + PSUM->SBUF move on DVE, two chunks
                # per op:  stage = (prob >= thr) * pm  [block bcast]
                nc.vector.scalar_tensor_tensor(
                    stages[g][:, rt, pos * CHW:(pos + 2) * CHW]
                    .rearrange("p (h b o) -> p h b o", h=2, o=CPT),
                    probs[rt][:, c0 * BPC:(c0 + 2) * BPC]
                    .rearrange("p (h b) -> p h b", h=2)
                    .broadcast_to([128, 2, BPC, CPT]),
                    thrs[rt],
                    pm[:, :, :CHW].rearrange(
                        "p h (b o) -> p h b o", o=CPT),
                    op0=OP.is_ge, op1=OP.mult)
            # Scale pipeline for the next pair (after this pair's applies).
            for cb in (c0 + 2, c0 + 3):
                if cb < NCH:
                    norm_back(cb)
            if g == NG - 1:
                # Final group: store per-pair so the tail drains early.
                nc.gpsimd.dma_start(
                    outv[:, :, c0 * CHW:(c0 + 2) * CHW],
                    stages[g][:, :, pos * CHW:(pos + 2) * CHW])
            elif pos == GRP - 2:
                nc.gpsimd.dma_start(
                    outv[:, :, g * GRP * CHW:(g + 1) * GRP * CHW],
                    stages[g][:])


def _build(k: int, inv_s2: float):
    nc = bacc.Bacc("TRN2", target_bir_lowering=False, debug=False)
    imgT_d = nc.dram_tensor(
        "imgT", [128, KD, RLOC], F32, kind="ExternalInput").ap()
    imgTh_d = nc.dram_tensor(
        "imgTh", [128, KD, RLOC], F16, kind="ExternalInput").ap()
    protoT_d = nc.dram_tensor(
        "protoT", [128, KD, NP], F32, kind="ExternalInput").ap()
    textT_d = nc.dram_tensor(
        "textT", [128, NG, KD, GRP * CHW], F16, kind="ExternalInput").ap()
    out_d = nc.dram_tensor(
        "out", [RLOC, NC], F16, kind="ExternalOutput").ap()
    with tile.TileContext(nc) as tc:
        with ExitStack() as ctx:
            _emit(ctx, tc, imgT_d, imgTh_d, protoT_d, textT_d, out_d,
                  k, inv_s2)
    nc.compile()
    return nc


def _tileT(a: np.ndarray) -> np.ndarray:
    """[N, D] -> [128, KD, N] with [p, kc, n] = a[n, kc*128 + p]."""
    n = a.shape[0]
    return np.ascontiguousarray(
        a.T.reshape(KD, 128, n).transpose(1, 0, 2))


def kernel(image_features, ima_proto, text_features_raw, logit_scale, num_test):
    global LAST_RESULTS
    img = np.ascontiguousarray(np.asarray(image_features, dtype=np.float32))
    proto = np.ascontiguousarray(np.asarray(ima_proto, dtype=np.float32))
    text = np.ascontiguousarray(np.asarray(text_features_raw, dtype=np.float32))
    assert img.shape == (B, D) and proto.shape == (NP, D) and text.shape == (NC, D)
    s = float(np.asarray(logit_scale))
    k = min(int(np.asarray(num_test)), NP)
    assert 1 <= k <= 16, f"kernel supports k in [1, 16], got {k}"
    inv_s2 = float(np.exp(-2.0 * s))

    nc = _build(k, inv_s2)

    # Host-side layout staging (transposes + dtype only; all math on device).
    protoT = _tileT(proto)                                   # [128, KD, 1000]
    textT4 = _tileT(text.astype(np.float16))                 # [128, KD, 10000]
    textT = np.ascontiguousarray(
        textT4.reshape(128, KD, NG, GRP * CHW).transpose(0, 2, 1, 3))
    in_maps = []
    for i in range(NCORES):
        imgT = _tileT(img[i * RLOC:(i + 1) * RLOC])          # [128, KD, 512]
        in_maps.append({
            "imgT": imgT,
            "imgTh": imgT.astype(np.float16),
            "protoT": protoT,
            "textT": textT,
        })
    trace = bool(int(os.environ.get("BASS_KERNEL_TRACE", "0")))
    res = run_bass_kernel_spmd(nc, in_maps, list(range(NCORES)), trace=trace)
    LAST_RESULTS = res
    return np.concatenate(
        [r["out"].astype(np.float32) for r in res.results], axis=0)


# revision 14
# speedup vs baseline: 1.3769x; 1.0543x over previous
"""Trainium2 Bass kernel for nn_CLIP topk_masking.

Computes, for full inputs (self-contained; shapes hardcoded):
    probability = image_features @ ima_proto.T          # [B, NP]
    thr_r       = k-th largest of probability row r
    sel[r, j]   = probability[r, j] >= thr_r            # top-k prototype mask
    text_n      = exp(logit_scale) * text_raw / ||text_raw||_row
    logits[r,c] = (image_features @ text_n.T)[r,c] * sel[r, c // 10]

Sharding: data-parallel over the batch axis across 8 NeuronCores
(rows 512/core); prototypes and text features replicated.

Design notes (v2 rewrite):
  - All matmul operands are staged HOST-SIDE in transposed, DMA-optimal
    tiled layouts (imgT / imgT_h / protoT / textT).  This removes every
    PE transpose and every PSUM->SBUF transpose copy of the previous
    version; the PE runs only the two real matmuls plus one tiny
    norm-reduction matmul per text chunk.
  - Text norms in the transposed layout: scalar engine squares each
    chunk (fp16), DVE combines the 4 contraction tiles, and a single
    ones[128,128] matmul both reduces over the last 128 partitions AND
    replicates norm^2 across partitions (PSUM).  Scalar does
    sqrt(norm2 * exp(-2s)); DVE reciprocal yields u_rep[128, 500] =
    exp(s)/||text_c|| replicated across partitions.
  - The mask is built on GPSIMD with one fused scalar_tensor_tensor per
    (row-tile, chunk): selxu = (prob >= thr) * u_rep.  This fuses the
    top-k compare, the block->class broadcast, and the norm scale.
  - DVE applies logits = pm * selxu straight out of PSUM into the fp16
    staging tile; SWDGE stores per 2000-column group (final group per
    chunk so the tail drains early).
  - Output is stored fp16 and upcast on the host.
"""

import os
from contextlib import ExitStack

import numpy as np

import concourse.bass as bass
import concourse.tile as tile
from concourse import bacc, mybir
from concourse.bass_utils import run_bass_kernel_spmd

# Problem shapes (hardcoded per contract).
B, D, NP, NC, CPT = 4096, 512, 1000, 10000, 10
NCORES = 8
RLOC = B // NCORES          # 512 rows per core
RT = RLOC // 128            # 4 row tiles per core
KD = D // 128               # 4 contraction chunks
CHW = 500                   # classes per chunk (matmul N; = 50 proto blocks)
NCH = NC // CHW             # 20 chunks
GRP = 4                     # chunks per group (per text-load / out-store DMA)
NG = NCH // GRP             # 5 groups
BPC = CHW // CPT            # 50 proto blocks per chunk
NEG = -1.0e30

F32 = mybir.dt.float32
F16 = mybir.dt.float16

LAST_RESULTS = None


def _emit(ctx: ExitStack, tc, imgT_d, imgTh_d, protoT_d, textT_d, out_d,
          k: int, inv_s2: float):
    nc = tc.nc
    AF = mybir.ActivationFunctionType
    OP = mybir.AluOpType

    const = ctx.enter_context(tc.tile_pool(name="const", bufs=1))
    persist = ctx.enter_context(tc.tile_pool(name="persist", bufs=1))

    ones_h = const.tile([128, 128], F16)
    nc.vector.memset(ones_h[:], 1.0)

    imgT = persist.tile([128, KD, RLOC], F32)
    imgTh = persist.tile([128, KD, RLOC], F16)
    protoT = persist.tile([128, KD, NP], F32)
    probs = [persist.tile([128, NP], F32, tag=f"prob{rt}", name=f"prob{rt}")
             for rt in range(RT)]

    # SWDGE load order: protoT + imgT first (phase-A critical path), then
    # text groups.  imgT_h rides the scalar HWDGE ring (disjoint engine
    # pressure).  protoT split in kc-halves so prob matmuls could start
    # on the first half while the second lands.
    nc.gpsimd.dma_start(protoT[:, :, :NP // 2], protoT_d[:, :, :NP // 2])
    nc.gpsimd.dma_start(imgT[:], imgT_d)
    nc.gpsimd.dma_start(protoT[:, :, NP // 2:], protoT_d[:, :, NP // 2:])
    nc.scalar.dma_start(imgTh[:], imgTh_d)

    pb_text = ctx.enter_context(tc.tile_pool(name="pb_text", bufs=3))
    pb_sq = ctx.enter_context(tc.tile_pool(name="pb_sq", bufs=3))
    pb_u = ctx.enter_context(tc.tile_pool(name="pb_u", bufs=4))
    pb_sx = ctx.enter_context(tc.tile_pool(name="pb_sx", bufs=4))
    pb_stage = ctx.enter_context(tc.tile_pool(name="pb_stage", bufs=2))
    pa_work = ctx.enter_context(tc.tile_pool(name="pa_work", bufs=2))
    pb_pace = ctx.enter_context(tc.tile_pool(name="pb_pace", bufs=2))

    tts = {}

    def pace_on(src_ap):
        # 1-element gpsimd read: delays subsequent SWDGE descriptor
        # generation until `src_ap`'s DMA completes, so earlier loads
        # keep all 16 DMA engines to themselves.
        pace = pb_pace.tile([1, 2], F32, tag="pace")
        nc.gpsimd.tensor_copy(pace[:].rearrange("a (b c) -> a b c", b=1), src_ap)

    def load_group(g: int):
        t_ = pb_text.tile([128, KD, GRP * CHW], F16, name=f"tt{g}", tag="tt")
        nc.gpsimd.dma_start(t_[:], textT_d[:, g])
        tts[g] = t_

    pace_on(protoT[0:1, 3:4, 0:2])
    pace_on(imgT[0:1, 0:1, 0:2])
    load_group(0)
    load_group(1)

    # ---------- Phase A: probability matmuls (f32) + per-row top-k thr ----
    # h-outer so the first half of protoT is enough to start the PE.
    thrs = [None] * RT
    with tc.tile_pool(name="pa_ps", bufs=3, space="PSUM") as pa_ps:
        for h in range(2):
            for rt in range(RT):
                ppr = pa_ps.tile([128, NP // 2], F32, tag="ppr")
                for kc in range(KD):
                    # fp32 (not fp16): ranking precision decides the mask.
                    nc.tensor.matmul(
                        ppr[:],
                        imgT[:, kc, rt * 128:(rt + 1) * 128],
                        protoT[:, kc, h * (NP // 2):(h + 1) * (NP // 2)],
                        start=(kc == 0), stop=(kc == KD - 1),
                    )
                nc.scalar.copy(
                    probs[rt][:, h * (NP // 2):(h + 1) * (NP // 2)], ppr[:])
                if h == 1:
                    # top-k threshold right behind each row-tile's prob so
                    # the first mask ops unblock as early as possible.
                    m8a = persist.tile([128, 8], F32, tag=f"m8a{rt}",
                                       name=f"m8a{rt}")
                    nc.vector.max(m8a[:], probs[rt][:])
                    if k <= 8:
                        thrs[rt] = m8a[:, k - 1:k]
                    else:
                        repl = pa_work.tile([128, NP], F32, tag="repl")
                        nc.vector.match_replace(
                            repl[:], m8a[:], probs[rt][:], NEG)
                        m8b = persist.tile([128, 8], F32, tag=f"m8b{rt}",
                                           name=f"m8b{rt}")
                        nc.vector.max(m8b[:], repl[:])
                        thrs[rt] = m8b[:, k - 9:k - 8]

    # ---------- Phase B: norms, logit matmuls, fused mask, store ----------
    # (sq/ttn inner dim padded to 512 so every fp16 kc slice is 4-byte
    # aligned: misaligned rhs drops the PE to single-pump rate.)
    with (
        tc.tile_pool(name="pb_psM", bufs=3, space="PSUM") as pb_psM,
        tc.tile_pool(name="pb_psN", bufs=2, space="PSUM") as pb_psN,
    ):
        sqs, nrps, ttns = {}, {}, {}

        def norm_front(c: int):
            # squares (scalar) + partition-reduce-and-replicate (PE):
            # nr[p, j] = norm^2 of text column j, replicated over p.
            g, pos = divmod(c, GRP)
            sq = pb_sq.tile([128, KD, 512], F16, tag="sq")
            nc.scalar.activation(
                sq[:, :, :CHW], tts[g][:, :, pos * CHW:(pos + 1) * CHW],
                AF.Square)
            nr = pb_psN.tile([128, CHW], F32, tag="nr")
            for kc in range(KD):
                nc.tensor.matmul(
                    nr[:], ones_h[:], sq[:, kc, :CHW],
                    start=(kc == 0), stop=(kc == KD - 1))
            sqs[c] = sq
            nrps[c] = nr

        def norm_back(c: int):
            # sqrt(norm2 * exp(-2s)) = ||t||/s; approx-reciprocal; scale tt.
            g, pos = divmod(c, GRP)
            del sqs[c]
            nr = nrps.pop(c)
            nrs = pb_u.tile([128, CHW], F32, tag="nrs")
            nc.scalar.activation(nrs[:], nr[:], AF.Sqrt, scale=inv_s2)
            ur = pb_u.tile([128, CHW], F32, tag="ur")
            nc.vector.reciprocal_approx_fast(ur[:], nrs[:])
            # fp16 copy of u: mixed f16*f32 tensor_tensor runs ~3x slower
            # on the DVE than f16*f16.
            urh = pb_u.tile([128, CHW], F16, tag="urh")
            nc.scalar.copy(urh[:], ur[:])
            ttn = pb_sx.tile([128, KD, 512], F16, tag="ttn")
            for kc in range(KD):
                eng = nc.vector if kc == 0 else nc.gpsimd
                eng.tensor_tensor(
                    ttn[:, kc, :CHW],
                    tts[g][:, kc, pos * CHW:(pos + 1) * CHW], urh[:],
                    op=OP.mult)
            ttns[c] = ttn

        stages = {}
        outv = out_d.rearrange("(t p) c -> p t c", p=128)
        norm_front(0)
        norm_front(1)
        norm_back(0)
        norm_back(1)
        for pr in range(NCH // 2):
            c0 = 2 * pr
            g, pos = divmod(c0, GRP)
            if pos == 0 and g + 2 < NG:
                pace_on(tts[g][0:1, 0:1, 0:2])
                load_group(g + 2)
            # Norm squares/reduce two pairs ahead of the PE.
            for cf in (c0 + 2, c0 + 3):
                if cf < NCH:
                    norm_front(cf)
            ttn0 = ttns.pop(c0)
            ttn1 = ttns.pop(c0 + 1)
            if pos == 0:
                stages[g] = pb_stage.tile(
                    [128, RT, GRP * CHW], F16, name=f"stg{g}", tag="stg")
            for rt in range(RT):
                pm = pb_psM.tile([128, 2, 512], F32, tag="pm")
                for side, ttn in ((0, ttn0), (1, ttn1)):
                    for kc in range(KD):
                        nc.tensor.matmul(
                            pm[:, side, :CHW],
                            imgTh[:, kc, rt * 128:(rt + 1) * 128],
                            ttn[:, kc, :CHW],
                            start=(kc == 0), stop=(kc == KD - 1),
                        )
                # Fused top-k mask + PSUM->SBUF move on DVE, two chunks
                # per op:  stage = (prob >= thr) * pm  [block bcast]
                nc.vector.scalar_tensor_tensor(
                    stages[g][:, rt, pos * CHW:(pos + 2) * CHW]
                    .rearrange("p (h b o) -> p h b o", h=2, o=CPT),
                    probs[rt][:, c0 * BPC:(c0 + 2) * BPC]
                    .rearrange("p (h b) -> p h b", h=2)
                    .broadcast_to([128, 2, BPC, CPT]),
                    thrs[rt],
                    pm[:, :, :CHW].rearrange(
                        "p h (b o) -> p h b o", o=CPT),
                    op0=OP.is_ge, op1=OP.mult)
                if g == NG - 1:
                    # Final group: store per (pair, rt) right behind each
                    # apply so the tail drains early.
                    nc.gpsimd.dma_start(
                        outv[:, rt:rt + 1, c0 * CHW:(c0 + 2) * CHW],
                        stages[g][:, rt:rt + 1, pos * CHW:(pos + 2) * CHW])
            # Scale pipeline for the next pair (after this pair's applies).
            for cb in (c0 + 2, c0 + 3):
                if cb < NCH:
                    norm_back(cb)
            if g != NG - 1 and pos == GRP - 2:
                nc.gpsimd.dma_start(
                    outv[:, :, g * GRP * CHW:(g + 1) * GRP * CHW],
                    stages[g][:])


def _build(k: int, inv_s2: float):
    nc = bacc.Bacc("TRN2", target_bir_lowering=False, debug=False)
    imgT_d = nc.dram_tensor(
        "imgT", [128, KD, RLOC], F32, kind="ExternalInput").ap()
    imgTh_d = nc.dram_tensor(
        "imgTh", [128, KD, RLOC], F16, kind="ExternalInput").ap()
    protoT_d = nc.dram_tensor(
        "protoT", [128, KD, NP], F32, kind="ExternalInput").ap()
    textT_d = nc.dram_tensor(
        "textT", [128, NG, KD, GRP * CHW], F16, kind="ExternalInput").ap()
    out_d = nc.dram_tensor(
        "out", [RLOC, NC], F16, kind="ExternalOutput").ap()
    with tile.TileContext(nc) as tc:
        with ExitStack() as ctx:
            _emit(ctx, tc, imgT_d, imgTh_d, protoT_d, textT_d, out_d,
                  k, inv_s2)
    nc.compile()
    return nc


def _tileT(a: np.ndarray) -> np.ndarray:
    """[N, D] -> [128, KD, N] with [p, kc, n] = a[n, kc*128 + p]."""
    n = a.shape[0]
    return np.ascontiguousarray(
        a.T.reshape(KD, 128, n).transpose(1, 0, 2))


def kernel(image_features, ima_proto, text_features_raw, logit_scale, num_test):
    global LAST_RESULTS
    img = np.ascontiguousarray(np.asarray(image_features, dtype=np.float32))
    proto = np.ascontiguousarray(np.asarray(ima_proto, dtype=np.float32))
    text = np.ascontiguousarray(np.asarray(text_features_raw, dtype=np.float32))
    assert img.shape == (B, D) and proto.shape == (NP, D) and text.shape == (NC, D)
    s = float(np.asarray(logit_scale))
    k = min(int(np.asarray(num_test)), NP)
    assert 1 <= k <= 16, f"kernel supports k in [1, 16], got {k}"
    inv_s2 = float(np.exp(-2.0 * s))

    nc = _build(k, inv_s2)

    # Host-side layout staging (transposes + dtype only; all math on device).
    protoT = _tileT(proto)                                   # [128, KD, 1000]
    textT4 = _tileT(text.astype(np.float16))                 # [128, KD, 10000]
    textT = np.ascontiguousarray(
        textT4.reshape(128, KD, NG, GRP * CHW).transpose(0, 2, 1, 3))
    in_maps = []
    for i in range(NCORES):
        imgT = _tileT(img[i * RLOC:(i + 1) * RLOC])          # [128, KD, 512]
        in_maps.append({
            "imgT": imgT,
            "imgTh": imgT.astype(np.float16),
            "protoT": protoT,
            "textT": textT,
        })
    trace = bool(int(os.environ.get("BASS_KERNEL_TRACE", "0")))
    res = run_bass_kernel_spmd(nc, in_maps, list(range(NCORES)), trace=trace)
    LAST_RESULTS = res
    return np.concatenate(
        [r["out"].astype(np.float32) for r in res.results], axis=0)


# revision 15
# speedup vs baseline: 1.4029x; 1.0189x over previous
"""Trainium2 Bass kernel for nn_CLIP topk_masking.

Computes, for full inputs (self-contained; shapes hardcoded):
    probability = image_features @ ima_proto.T          # [B, NP]
    thr_r       = k-th largest of probability row r
    sel[r, j]   = probability[r, j] >= thr_r            # top-k prototype mask
    text_n      = exp(logit_scale) * text_raw / ||text_raw||_row
    logits[r,c] = (image_features @ text_n.T)[r,c] * sel[r, c // 10]

Sharding: data-parallel over the batch axis across 8 NeuronCores
(rows 512/core); prototypes and text features replicated.

Design notes (v2 rewrite):
  - All matmul operands are staged HOST-SIDE in transposed, DMA-optimal
    tiled layouts (imgT / imgT_h / protoT / textT).  This removes every
    PE transpose and every PSUM->SBUF transpose copy of the previous
    version; the PE runs only the two real matmuls plus one tiny
    norm-reduction matmul per text chunk.
  - Text norms in the transposed layout: scalar engine squares each
    chunk (fp16), DVE combines the 4 contraction tiles, and a single
    ones[128,128] matmul both reduces over the last 128 partitions AND
    replicates norm^2 across partitions (PSUM).  Scalar does
    sqrt(norm2 * exp(-2s)); DVE reciprocal yields u_rep[128, 500] =
    exp(s)/||text_c|| replicated across partitions.
  - The mask is built on GPSIMD with one fused scalar_tensor_tensor per
    (row-tile, chunk): selxu = (prob >= thr) * u_rep.  This fuses the
    top-k compare, the block->class broadcast, and the norm scale.
  - DVE applies logits = pm * selxu straight out of PSUM into the fp16
    staging tile; SWDGE stores per 2000-column group (final group per
    chunk so the tail drains early).
  - Output is stored fp16 and upcast on the host.
"""

import os
from contextlib import ExitStack

import numpy as np

import concourse.bass as bass
import concourse.tile as tile
from concourse import bacc, mybir
from concourse.bass_utils import run_bass_kernel_spmd

# Problem shapes (hardcoded per contract).
B, D, NP, NC, CPT = 4096, 512, 1000, 10000, 10
NCORES = 8
RLOC = B // NCORES          # 512 rows per core
RT = RLOC // 128            # 4 row tiles per core
KD = D // 128               # 4 contraction chunks
CHW = 500                   # classes per chunk (matmul N; = 50 proto blocks)
NCH = NC // CHW             # 20 chunks
GRP = 4                     # chunks per group (per text-load / out-store DMA)
NG = NCH // GRP             # 5 groups
BPC = CHW // CPT            # 50 proto blocks per chunk
NEG = -1.0e30

F32 = mybir.dt.float32
F16 = mybir.dt.float16

LAST_RESULTS = None


def _emit(ctx: ExitStack, tc, imgT_d, imgTh_d, protoT_d, textT_d, out_d,
          k: int, inv_s2: float):
    nc = tc.nc
    AF = mybir.ActivationFunctionType
    OP = mybir.AluOpType

    const = ctx.enter_context(tc.tile_pool(name="const", bufs=1))
    persist = ctx.enter_context(tc.tile_pool(name="persist", bufs=1))

    ones_h = const.tile([128, 128], F16)
    nc.vector.memset(ones_h[:], 1.0)

    imgT = persist.tile([128, KD, RLOC], F32)
    imgTh = persist.tile([128, KD, RLOC], F16)
    protoT = persist.tile([128, KD, NP], F32)
    probs = [persist.tile([128, NP], F32, tag=f"prob{rt}", name=f"prob{rt}")
             for rt in range(RT)]

    # SWDGE load order: protoT + imgT first (phase-A critical path), then
    # text groups.  imgT_h rides the scalar HWDGE ring (disjoint engine
    # pressure).  protoT split in kc-halves so prob matmuls could start
    # on the first half while the second lands.
    nc.gpsimd.dma_start(protoT[:, :, :NP // 2], protoT_d[:, :, :NP // 2])
    nc.gpsimd.dma_start(imgT[:], imgT_d)
    nc.gpsimd.dma_start(protoT[:, :, NP // 2:], protoT_d[:, :, NP // 2:])
    nc.scalar.dma_start(imgTh[:], imgTh_d)

    pb_text = ctx.enter_context(tc.tile_pool(name="pb_text", bufs=3))
    pb_sq = ctx.enter_context(tc.tile_pool(name="pb_sq", bufs=3))
    pb_u = ctx.enter_context(tc.tile_pool(name="pb_u", bufs=4))
    pb_sx = ctx.enter_context(tc.tile_pool(name="pb_sx", bufs=4))
    pb_stage = ctx.enter_context(tc.tile_pool(name="pb_stage", bufs=2))
    pa_work = ctx.enter_context(tc.tile_pool(name="pa_work", bufs=2))
    pb_pace = ctx.enter_context(tc.tile_pool(name="pb_pace", bufs=2))

    tts = {}

    def pace_on(src_ap):
        # 1-element gpsimd read: delays subsequent SWDGE descriptor
        # generation until `src_ap`'s DMA completes, so earlier loads
        # keep all 16 DMA engines to themselves.
        pace = pb_pace.tile([1, 2], F32, tag="pace")
        nc.gpsimd.tensor_copy(pace[:].rearrange("a (b c) -> a b c", b=1), src_ap)

    def load_group(g: int):
        t_ = pb_text.tile([128, KD, GRP, 512], F16, name=f"tt{g}", tag="tt")
        nc.gpsimd.dma_start(t_[:], textT_d[:, g])
        tts[g] = t_

    pace_on(protoT[0:1, 3:4, 0:2])
    pace_on(imgT[0:1, 0:1, 0:2])
    load_group(0)
    load_group(1)

    # ---------- Phase A: probability matmuls (f32) + per-row top-k thr ----
    # h-outer so the first half of protoT is enough to start the PE.
    thrs = [None] * RT
    with tc.tile_pool(name="pa_ps", bufs=3, space="PSUM") as pa_ps:
        for h in range(2):
            for rt in range(RT):
                ppr = pa_ps.tile([128, NP // 2], F32, tag="ppr")
                for kc in range(KD):
                    # fp32 (not fp16): ranking precision decides the mask.
                    nc.tensor.matmul(
                        ppr[:],
                        imgT[:, kc, rt * 128:(rt + 1) * 128],
                        protoT[:, kc, h * (NP // 2):(h + 1) * (NP // 2)],
                        start=(kc == 0), stop=(kc == KD - 1),
                    )
                nc.scalar.copy(
                    probs[rt][:, h * (NP // 2):(h + 1) * (NP // 2)], ppr[:])
                if h == 1:
                    # top-k threshold right behind each row-tile's prob so
                    # the first mask ops unblock as early as possible.
                    m8a = persist.tile([128, 8], F32, tag=f"m8a{rt}",
                                       name=f"m8a{rt}")
                    nc.vector.max(m8a[:], probs[rt][:])
                    if k <= 8:
                        thrs[rt] = m8a[:, k - 1:k]
                    else:
                        repl = pa_work.tile([128, NP], F32, tag="repl")
                        nc.vector.match_replace(
                            repl[:], m8a[:], probs[rt][:], NEG)
                        m8b = persist.tile([128, 8], F32, tag=f"m8b{rt}",
                                           name=f"m8b{rt}")
                        nc.vector.max(m8b[:], repl[:])
                        thrs[rt] = m8b[:, k - 9:k - 8]

    # ---------- Phase B: norms, logit matmuls, fused mask, store ----------
    # (sq/ttn inner dim padded to 512 so every fp16 kc slice is 4-byte
    # aligned: misaligned rhs drops the PE to single-pump rate.)
    with (
        tc.tile_pool(name="pb_psM", bufs=3, space="PSUM") as pb_psM,
        tc.tile_pool(name="pb_psN", bufs=2, space="PSUM") as pb_psN,
    ):
        sqs, nrps, ttns = {}, {}, {}

        def norm_front(c: int):
            # squares (scalar) + partition-reduce-and-replicate (PE):
            # nr[p, j] = norm^2 of text column j, replicated over p.
            g, pos = divmod(c, GRP)
            sq = pb_sq.tile([128, KD, 512], F16, tag="sq")
            nc.scalar.activation(
                sq[:, :, :CHW], tts[g][:, :, pos, :CHW],
                AF.Square)
            nr = pb_psN.tile([128, CHW], F32, tag="nr")
            for kc in range(KD):
                nc.tensor.matmul(
                    nr[:], ones_h[:], sq[:, kc, :CHW],
                    start=(kc == 0), stop=(kc == KD - 1))
            sqs[c] = sq
            nrps[c] = nr

        def norm_back(c: int):
            # sqrt(norm2 * exp(-2s)) = ||t||/s; approx-reciprocal; scale tt.
            g, pos = divmod(c, GRP)
            del sqs[c]
            nr = nrps.pop(c)
            nrs = pb_u.tile([128, CHW], F32, tag="nrs")
            nc.scalar.activation(nrs[:], nr[:], AF.Sqrt, scale=inv_s2)
            ur = pb_u.tile([128, CHW], F32, tag="ur")
            nc.vector.reciprocal_approx_fast(ur[:], nrs[:])
            # fp16 copy of u: mixed f16*f32 tensor_tensor runs ~3x slower
            # on the DVE than f16*f16.
            urh = pb_u.tile([128, CHW], F16, tag="urh")
            nc.scalar.copy(urh[:], ur[:])
            ttn = pb_sx.tile([128, KD, 512], F16, tag="ttn")
            for kc in range(KD):
                eng = nc.vector if kc == 0 else nc.gpsimd
                eng.tensor_tensor(
                    ttn[:, kc, :CHW],
                    tts[g][:, kc, pos, :CHW], urh[:],
                    op=OP.mult)
            ttns[c] = ttn

        stages = {}
        outv = out_d.rearrange("(t p) c -> p t c", p=128)
        norm_front(0)
        norm_front(1)
        norm_back(0)
        norm_back(1)
        for pr in range(NCH // 2):
            c0 = 2 * pr
            g, pos = divmod(c0, GRP)
            if pos == 0 and g + 2 < NG:
                pace_on(tts[g][0:1, 0:1, 0:1, 0:2])
                load_group(g + 2)
            # Norm squares/reduce two pairs ahead of the PE.
            for cf in (c0 + 2, c0 + 3):
                if cf < NCH:
                    norm_front(cf)
            ttn0 = ttns.pop(c0)
            ttn1 = ttns.pop(c0 + 1)
            if pos == 0:
                stages[g] = pb_stage.tile(
                    [128, RT, GRP * CHW], F16, name=f"stg{g}", tag="stg")
            for rt in range(RT):
                pm = pb_psM.tile([128, 2, 512], F32, tag="pm")
                for side, ttn in ((0, ttn0), (1, ttn1)):
                    for kc in range(KD):
                        nc.tensor.matmul(
                            pm[:, side, :CHW],
                            imgTh[:, kc, rt * 128:(rt + 1) * 128],
                            ttn[:, kc, :CHW],
                            start=(kc == 0), stop=(kc == KD - 1),
                        )
                # Fused top-k mask + PSUM->SBUF move on DVE, two chunks
                # per op:  stage = (prob >= thr) * pm  [block bcast]
                nc.vector.scalar_tensor_tensor(
                    stages[g][:, rt, pos * CHW:(pos + 2) * CHW]
                    .rearrange("p (h b o) -> p h b o", h=2, o=CPT),
                    probs[rt][:, c0 * BPC:(c0 + 2) * BPC]
                    .rearrange("p (h b) -> p h b", h=2)
                    .broadcast_to([128, 2, BPC, CPT]),
                    thrs[rt],
                    pm[:, :, :CHW].rearrange(
                        "p h (b o) -> p h b o", o=CPT),
                    op0=OP.is_ge, op1=OP.mult)
                if g == NG - 1:
                    # Final group: store per (pair, rt) right behind each
                    # apply, on the idle sync-engine HWDGE ring so the
                    # tail never waits on SWDGE descriptor generation.
                    nc.sync.dma_start(
                        outv[:, rt:rt + 1, c0 * CHW:(c0 + 2) * CHW],
                        stages[g][:, rt:rt + 1, pos * CHW:(pos + 2) * CHW])
            # Scale pipeline for the next pair (after this pair's applies).
            for cb in (c0 + 2, c0 + 3):
                if cb < NCH:
                    norm_back(cb)
            if g != NG - 1 and pos == GRP - 2:
                nc.gpsimd.dma_start(
                    outv[:, :, g * GRP * CHW:(g + 1) * GRP * CHW],
                    stages[g][:])


def _build(k: int, inv_s2: float):
    nc = bacc.Bacc("TRN2", target_bir_lowering=False, debug=False)
    imgT_d = nc.dram_tensor(
        "imgT", [128, KD, RLOC], F32, kind="ExternalInput").ap()
    imgTh_d = nc.dram_tensor(
        "imgTh", [128, KD, RLOC], F16, kind="ExternalInput").ap()
    protoT_d = nc.dram_tensor(
        "protoT", [128, KD, NP], F32, kind="ExternalInput").ap()
    textT_d = nc.dram_tensor(
        "textT", [128, NG, KD, GRP, 512], F16, kind="ExternalInput").ap()
    out_d = nc.dram_tensor(
        "out", [RLOC, NC], F16, kind="ExternalOutput").ap()
    with tile.TileContext(nc) as tc:
        with ExitStack() as ctx:
            _emit(ctx, tc, imgT_d, imgTh_d, protoT_d, textT_d, out_d,
                  k, inv_s2)
    nc.compile()
    return nc


def _tileT(a: np.ndarray) -> np.ndarray:
    """[N, D] -> [128, KD, N] with [p, kc, n] = a[n, kc*128 + p]."""
    n = a.shape[0]
    return np.ascontiguousarray(
        a.T.reshape(KD, 128, n).transpose(1, 0, 2))


def kernel(image_features, ima_proto, text_features_raw, logit_scale, num_test):
    global LAST_RESULTS
    img = np.ascontiguousarray(np.asarray(image_features, dtype=np.float32))
    proto = np.ascontiguousarray(np.asarray(ima_proto, dtype=np.float32))
    text = np.ascontiguousarray(np.asarray(text_features_raw, dtype=np.float32))
    assert img.shape == (B, D) and proto.shape == (NP, D) and text.shape == (NC, D)
    s = float(np.asarray(logit_scale))
    k = min(int(np.asarray(num_test)), NP)
    assert 1 <= k <= 16, f"kernel supports k in [1, 16], got {k}"
    inv_s2 = float(np.exp(-2.0 * s))

    nc = _build(k, inv_s2)

    # Host-side layout staging (transposes + dtype only; all math on device).
    protoT = _tileT(proto)                                   # [128, KD, 1000]
    textT4 = _tileT(text.astype(np.float16))                 # [128, KD, 10000]
    textT = np.zeros((128, NG, KD, GRP, 512), dtype=np.float16)
    textT[:, :, :, :, :CHW] = textT4.reshape(
        128, KD, NG, GRP, CHW).transpose(0, 2, 1, 3, 4)
    in_maps = []
    for i in range(NCORES):
        imgT = _tileT(img[i * RLOC:(i + 1) * RLOC])          # [128, KD, 512]
        in_maps.append({
            "imgT": imgT,
            "imgTh": imgT.astype(np.float16),
            "protoT": protoT,
            "textT": textT,
        })
    trace = bool(int(os.environ.get("BASS_KERNEL_TRACE", "0")))
    res = run_bass_kernel_spmd(nc, in_maps, list(range(NCORES)), trace=trace)
    LAST_RESULTS = res
    return np.concatenate(
        [r["out"].astype(np.float32) for r in res.results], axis=0)
